# revision 1
# baseline (speedup 1.0000x reference)
"""GAU (gated attention unit) Trainium2 Bass kernel.

Sharding: 8 cores = 4 batches x 2 E-halves.
  core c -> batch b = c//2, E-half h = c%2 (cols h*768:(h+1)*768 of E=1536).
Each core computes, for its batch:
  LN stats, z/q/k (S=128, shared across E), u/v for its E-half,
  qk^T = k @ q^T, a^T = relu(qk^T)^2 (mask folded into gamma_k/beta_k),
  attn^T = v^T @ a^T, g = u^T * attn^T, out_partial = g^T @ Wo_half.
Host: out[b] = part[2b] + part[2b+1] + bo + x[b].

Precision: all matmuls use fp16 operands with fp32 PSUM accumulation (fp16
streams 1 cycle/row on the PE and enables fast, hidden weight loads;
fp32/fp32r stream at 2-4 cycles/row with slow unhidden LDWEIGHTS). LN stats
and the q/k affine run in fp32. Measured end-to-end relative error ~7e-4 vs
the fp32 reference; all fp16-stored intermediates (max |g| ~12k) stay well
inside fp16 range.

LayerNorm without transposes or a pre-scaled copy of x: matmuls consume raw
fp16 x directly (so they never wait on the stats), the mean term is folded
in as a rank-2 PSUM correction [-colsum(W); bias] x [mu; 1/rstd], and the
rstd scale is applied during PSUM evacuation (DVE tensor op in E-major,
ACT copy-with-scale in token-major). Stats come from bn_stats/bn_aggr on
token-major x tiles; mu/rstd rows are built with tiny PE transposes and a
DRAM-bounce partition broadcast, all off the matmul critical path.
Measured HW exec time: ~269 us per core (8 cores in parallel).
"""

import numpy as np
from contextlib import ExitStack

import concourse.bass as bass
import concourse.tile as tile
from concourse import bacc, mybir
from concourse.bass_utils import run_bass_kernel_spmd
from concourse.masks import make_identity

# Problem dims (hardcoded per the task contract)
B, T, D, S, E = 4, 2048, 768, 128, 1536
EH = E // 2          # per-core E half
P = 128
ND = D // P          # 6 d-chunks
NE = EH // P         # 6 e-chunks
NT = T // P          # 16 token chunks
TT = 512             # t-tile (phase B) and attention t-block
NTT = T // TT        # 4
LN_EPS = 1e-5

F32 = mybir.dt.float32
FP16 = mybir.dt.float16
AF = mybir.ActivationFunctionType
ALU = mybir.AluOpType
NPFP16 = np.float16

N_CORES = 8


def build_module():
    nc = bacc.Bacc("TRN2", debug=False, num_devices=N_CORES, num_swdge_queues=4)

    # ---- DRAM I/O ----
    xT_d = nc.dram_tensor("xT", [D, T], FP16, kind="ExternalInput").ap()
    x_d = nc.dram_tensor("x", [T, D], FP16, kind="ExternalInput").ap()
    gq_d = nc.dram_tensor("gqT", [S, T], FP16, kind="ExternalInput").ap()
    bq_d = nc.dram_tensor("bqT", [S, T], FP16, kind="ExternalInput").ap()
    gk_d = nc.dram_tensor("gkT", [S, T], FP16, kind="ExternalInput").ap()
    bk_d = nc.dram_tensor("bkT", [S, T], FP16, kind="ExternalInput").ap()
    wz_d = nc.dram_tensor("Wz", [D, S], FP16, kind="ExternalInput").ap()
    wu_d = nc.dram_tensor("Wu", [D, EH], FP16, kind="ExternalInput").ap()
    wv_d = nc.dram_tensor("Wv", [D, EH], FP16, kind="ExternalInput").ap()
    wo_d = nc.dram_tensor("Wo", [EH, D], FP16, kind="ExternalInput").ap()
    cz_d = nc.dram_tensor("Cz", [2, S], FP16, kind="ExternalInput").ap()
    cu_d = nc.dram_tensor("Cu", [2, EH], FP16, kind="ExternalInput").ap()
    cv_d = nc.dram_tensor("Cv", [2, EH], FP16, kind="ExternalInput").ap()
    out_d = nc.dram_tensor("outp", [T, D], F32, kind="ExternalOutput").ap()
    # scratch row for the rstd partition-broadcast bounce
    scr_d = nc.dram_tensor("rstd_scr", [1, T], F32, kind="Internal").ap()

    xT_r = xT_d.rearrange("(c p) t -> p c t", p=P)   # [128, 6, 2048]
    wu_r = wu_d.rearrange("(c p) e -> p c e", p=P)   # [128, 6, 768]
    wv_r = wv_d.rearrange("(c p) e -> p c e", p=P)
    wz_r = wz_d.rearrange("(c p) s -> p c s", p=P)   # [128, 6, 128]
    wo_r = wo_d.rearrange("(c p) d -> p c d", p=P)   # [128, 6, 768]

    with tile.TileContext(nc) as tc, ExitStack() as ctx:
        # ---------- persistent pools ----------
        persist = ctx.enter_context(tc.tile_pool(name="persist", bufs=1))
        ident = persist.tile([P, P], F32)
        make_identity(nc, ident)
        eps_t = persist.tile([P, 1], F32)
        nc.vector.memset(eps_t, LN_EPS)
        # prefetch ACT tables off the critical path
        warm = persist.tile([P, 1], F32)
        nc.scalar.activation(out=warm, in_=eps_t, func=AF.Sqrt)
        nc.scalar.activation(out=warm, in_=warm, func=AF.Relu)
        # S3 rows (fp32): 0 = mu, 1 = 1/rstd, 2 = rstd.
        # S2b (fp16 copy of rows 0:2) is the rank-2 matmul operand: the PSUM
        # correction is [-colsum(W); bias] x [mu; 1/rstd]; the whole PSUM is
        # then scaled by rstd at evacuation, yielding ((x-mu)@W)*rstd + bias.
        S3 = persist.tile([3, T], F32)
        S2b = persist.tile([2, T], FP16)
        rstd_b = persist.tile([P, T], F32)           # rstd broadcast to 128 parts
        rstd_col = persist.tile([P, NT], F32)        # token-major rstd columns
        qT = persist.tile([S, T], FP16)
        kT = persist.tile([S, T], FP16)
        uT = persist.tile([P, NE, T], FP16)          # 12KB/part
        v_t = persist.tile([P, NT, EH], FP16)        # 12KB/part
        wo_t = persist.tile([P, NE, D], FP16)
        ps = ctx.enter_context(tc.tile_pool(name="ps", bufs=6, space="PSUM"))
        atp = ctx.enter_context(tc.tile_pool(name="atp", bufs=2))

        # ---------- phase B: stats + z/q/k + u + v, per t-tile ----------
        with (
            tc.tile_pool(name="statw", bufs=3) as sw,
            tc.tile_pool(name="statp", bufs=2, space="PSUM") as sp,
            tc.tile_pool(name="w1", bufs=1) as w1,
            tc.tile_pool(name="b1w", bufs=2) as b1w,
        ):
            wz_t = w1.tile([P, ND, S], FP16)
            nc.sync.dma_start(out=wz_t, in_=wz_r)
            cz_t = w1.tile([2, S], FP16)
            nc.sync.dma_start(out=cz_t, in_=cz_d)
            cu_t = w1.tile([2, EH], FP16)
            nc.sync.dma_start(out=cu_t, in_=cu_d)
            cv_t = w1.tile([2, EH], FP16)
            nc.sync.dma_start(out=cv_t, in_=cv_d)
            wu_t = w1.tile([P, ND, EH], FP16)
            wv_t = w1.tile([P, ND, EH], FP16)
            for c in range(ND):
                nc.gpsimd.dma_start(out=wu_t[:, c, :], in_=wu_r[:, c, :])
                nc.gpsimd.dma_start(out=wv_t[:, c, :], in_=wv_r[:, c, :])
            nc.gpsimd.dma_start(out=wo_t, in_=wo_r)

            for tt in range(NTT):
                ts_ = slice(tt * TT, (tt + 1) * TT)
                # --- raw x block (fp16, feature-major) ---
                xb = b1w.tile([P, ND, TT], FP16, tag="xb", bufs=3)
                for c in range(ND):
                    nc.sync.dma_start(out=xb[:, c, :], in_=xT_r[:, c, ts_])

                # --- LN stats for the 4 token chunks of this t-tile ---
                for sub in range(TT // P):
                    it = tt * (TT // P) + sub
                    xt = sw.tile([P, D], FP16, tag="xtile")
                    nc.sync.dma_start(out=xt, in_=x_d[it * P:(it + 1) * P, :])
                    st = sw.tile([P, 3, 6], F32, tag="bnst")
                    for g in range(3):
                        nc.vector.bn_stats(
                            out=st[:, g, :], in_=xt[:, g * 256:(g + 1) * 256]
                        )
                    mv = sw.tile([P, 2], F32, tag="mv")
                    nc.vector.bn_aggr(out=mv, in_=st)
                    # pair cols: 0 = mu, 1 = 1/rstd = sqrt(var+eps), 2 = rstd
                    pair = sw.tile([P, 3], F32, tag="pair")
                    nc.vector.tensor_copy(out=pair[:, 0:1], in_=mv[:, 0:1])
                    nc.scalar.activation(
                        out=pair[:, 1:2], in_=mv[:, 1:2], func=AF.Sqrt,
                        bias=eps_t, scale=1.0,
                    )
                    nc.vector.reciprocal(out=pair[:, 2:3], in_=pair[:, 1:2])
                    nc.vector.tensor_copy(
                        out=rstd_col[:, it:it + 1], in_=pair[:, 2:3]
                    )
                    pt = sp.tile([3, P], F32, tag="pt")
                    nc.tensor.transpose(pt, pair, ident)
                    nc.vector.tensor_copy(
                        out=S3[:, it * P:(it + 1) * P], in_=pt
                    )
                # fp16 rank-2 operand rows [mu; 1/rstd]
                nc.vector.tensor_copy(out=S2b[:, ts_], in_=S3[0:2, ts_])
                # broadcast rstd to all partitions via a DRAM bounce
                # (off the matmul critical path)
                nc.sync.dma_start(out=scr_d[:, ts_], in_=S3[2:3, ts_])
                bcast_src = bass.AP(
                    tensor=scr_d.tensor, offset=scr_d.offset + tt * TT,
                    ap=[[0, P], [1, TT]],
                )
                nc.sync.dma_start(out=rstd_b[:, ts_], in_=bcast_src)

                # --- z -> q,k ---
                zp = ps.tile([S, TT], F32, tag="mm")
                for c in range(ND):
                    nc.tensor.matmul(
                        zp, wz_t[:, c, :], xb[:, c, :],
                        start=(c == 0), stop=False,
                    )
                nc.tensor.matmul(zp, cz_t, S2b[:, ts_], start=False, stop=True)
                gq = b1w.tile([S, TT], FP16, tag="gq")
                nc.sync.dma_start(out=gq, in_=gq_d[:, ts_])
                bq = b1w.tile([S, TT], FP16, tag="bq")
                nc.sync.dma_start(out=bq, in_=bq_d[:, ts_])
                gk = b1w.tile([S, TT], FP16, tag="gk")
                nc.sync.dma_start(out=gk, in_=gk_d[:, ts_])
                bk = b1w.tile([S, TT], FP16, tag="bk")
                nc.sync.dma_start(out=bk, in_=bk_d[:, ts_])
                # z = zp*rstd (shared), then q/k = z*gamma + beta
                zs = b1w.tile([S, TT], F32, tag="zs")
                nc.vector.tensor_mul(out=zs, in0=zp, in1=rstd_b[:S, ts_])
                qf = b1w.tile([S, TT], F32, tag="qf")
                nc.vector.tensor_mul(out=qf, in0=zs, in1=gq)
                nc.vector.tensor_add(out=qT[:, ts_], in0=qf, in1=bq)
                kf = b1w.tile([S, TT], F32, tag="kf")
                nc.vector.tensor_mul(out=kf, in0=zs, in1=gk)
                nc.vector.tensor_add(out=kT[:, ts_], in0=kf, in1=bk)

                # --- u (E-major): evac applies rstd ---
                for e in range(NE):
                    up = ps.tile([P, TT], F32, tag="mm")
                    for c in range(ND):
                        nc.tensor.matmul(
                            up, wu_t[:, c, e * P:(e + 1) * P], xb[:, c, :],
                            start=(c == 0), stop=False,
                        )
                    nc.tensor.matmul(
                        up, cu_t[:, e * P:(e + 1) * P], S2b[:, ts_],
                        start=False, stop=True,
                    )
                    nc.vector.tensor_mul(
                        out=uT[:, e, ts_], in0=up, in1=rstd_b[:, ts_]
                    )

                # --- v (token-major): evac applies rstd per-partition ---
                for tch in range(TT // P):
                    it = tt * (TT // P) + tch
                    tc_ = slice(it * P, (it + 1) * P)
                    for (e0, ew) in ((0, 384), (384, 384)):
                        vp = ps.tile([P, 384], F32, tag="mm")
                        for c in range(ND):
                            nc.tensor.matmul(
                                vp, xb[:, c, tch * P:(tch + 1) * P],
                                wv_t[:, c, e0:e0 + ew],
                                start=(c == 0), stop=False,
                            )
                        nc.tensor.matmul(
                            vp, S2b[:, tc_], cv_t[:, e0:e0 + ew],
                            start=False, stop=True,
                        )
                        nc.scalar.activation(
                            out=v_t[:, it, e0:e0 + ew], in_=vp,
                            func=AF.Copy, scale=rstd_col[:, it:it + 1],
                        )

        # ---------- phase C: attention + output ----------
        with (
            tc.tile_pool(name="c3w", bufs=3) as c3w,
            tc.tile_pool(name="ps2", bufs=2, space="PSUM") as ps2,
        ):
            for tb in range(NTT):
                tbs = slice(tb * TT, (tb + 1) * TT)
                aT = atp.tile([P, NT, TT], FP16, tag="aT")
                for uc in range(NT):
                    qk = ps2.tile([P, TT], F32, tag="qk")
                    nc.tensor.matmul(
                        qk, kT[:, uc * P:(uc + 1) * P], qT[:, tbs],
                        start=True, stop=True,
                    )
                    # a = relu(qk)^2: ACT does relu (psum->fp16),
                    # DVE squares in fp16 (2x mode)
                    rt = c3w.tile([P, TT], FP16, tag="rt")
                    nc.scalar.activation(out=rt, in_=qk, func=AF.Relu)
                    nc.vector.tensor_mul(out=aT[:, uc, :], in0=rt, in1=rt)
                for e in range(NE):
                    at_ps = ps.tile([P, TT], F32, tag="mm")
                    for uc in range(NT):
                        nc.tensor.matmul(
                            at_ps, v_t[:, uc, e * P:(e + 1) * P],
                            aT[:, uc, :],
                            start=(uc == 0), stop=(uc == NT - 1),
                        )
                    # g = u * attn, in place over uT (fp16)
                    nc.vector.tensor_mul(
                        out=uT[:, e, tbs], in0=at_ps, in1=uT[:, e, tbs]
                    )
                for tch in range(TT // P):
                    it = tb * (TT // P) + tch
                    tc_ = slice(it * P, (it + 1) * P)
                    osb = c3w.tile([P, D], F32, tag="osb")
                    for (d0, dw) in ((0, 384), (384, 384)):
                        op_ = ps.tile([P, 384], F32, tag="mm")
                        for e in range(NE):
                            nc.tensor.matmul(
                                op_, uT[:, e, tc_], wo_t[:, e, d0:d0 + dw],
                                start=(e == 0), stop=(e == NE - 1),
                            )
                        nc.scalar.copy(out=osb[:, d0:d0 + dw], in_=op_)
                    nc.gpsimd.dma_start(out=out_d[tc_, :], in_=osb)

    nc.finalize()
    return nc


def prep_core_inputs(inputs):
    """Host-side slicing: returns the list of 8 per-core input maps."""
    f = np.float32
    x = np.asarray(inputs["x"], f)
    mask = np.asarray(inputs["mask"])
    ln_w = np.asarray(inputs["ln_w"], f)
    ln_b = np.asarray(inputs["ln_b"], f)
    Wz = np.asarray(inputs["Wz"], f)
    bz = np.asarray(inputs["bz"], f)
    Wu = np.asarray(inputs["Wu"], f)
    bu = np.asarray(inputs["bu"], f)
    Wv = np.asarray(inputs["Wv"], f)
    bv = np.asarray(inputs["bv"], f)
    Wo = np.asarray(inputs["Wo"], f)
    gq = np.asarray(inputs["gamma_q"], f)
    bq = np.asarray(inputs["beta_q"], f)
    gk = np.asarray(inputs["gamma_k"], f)
    bk = np.asarray(inputs["beta_k"], f)

    # fold ln_w into the weights, ln_b into the matmul biases
    Wz_e = np.ascontiguousarray(ln_w[:, None] * Wz)
    Wu_e = ln_w[:, None] * Wu
    Wv_e = ln_w[:, None] * Wv
    bz_e = ln_b @ Wz + bz
    bu_e = ln_b @ Wu + bu
    bv_e = ln_b @ Wv + bv

    gqT = np.ascontiguousarray(gq.T.astype(NPFP16))
    bqT = np.ascontiguousarray(bq.T.astype(NPFP16))
    gkT = gk.T.astype(np.float32)
    bkT = bk.T.astype(np.float32)
    # C rows pair with S2b rows: row0 <-> mu (-colsum), row1 <-> 1/rstd (bias)
    Cz = np.stack([-Wz_e.sum(0), bz_e]).astype(NPFP16)

    in_maps = []
    for c in range(N_CORES):
        b, h = c // 2, c % 2
        cols = slice(h * EH, (h + 1) * EH)
        keep = (~mask[b]).astype(f)  # 1 = attend, 0 = masked-out key
        Wu_h = Wu_e[:, cols]
        Wv_h = Wv_e[:, cols]
        in_maps.append({
            "x": np.ascontiguousarray(x[b].astype(NPFP16)),
            "xT": np.ascontiguousarray(x[b].T.astype(NPFP16)),
            "gqT": gqT,
            "bqT": bqT,
            "gkT": np.ascontiguousarray((gkT * keep[None, :]).astype(NPFP16)),
            "bkT": np.ascontiguousarray((bkT * keep[None, :]).astype(NPFP16)),
            "Wz": Wz_e.astype(NPFP16),
            "Wu": np.ascontiguousarray(Wu_h.astype(NPFP16)),
            "Wv": np.ascontiguousarray(Wv_h.astype(NPFP16)),
            "Wo": np.ascontiguousarray(Wo[cols, :].astype(NPFP16)),
            "Cz": Cz,
            "Cu": np.ascontiguousarray(
                np.stack([-Wu_h.sum(0), bu_e[cols]]).astype(NPFP16)),
            "Cv": np.ascontiguousarray(
                np.stack([-Wv_h.sum(0), bv_e[cols]]).astype(NPFP16)),
        })
    return in_maps


def combine_outputs(inputs, parts):
    """parts: list of 8 [T, D] partial outputs -> full [B, T, D]."""
    f = np.float32
    x = np.asarray(inputs["x"], f)
    bo = np.asarray(inputs["bo"], f)
    out = np.empty((B, T, D), f)
    for b in range(B):
        out[b] = parts[2 * b] + parts[2 * b + 1] + bo[None, :] + x[b]
    return out


_NC_CACHE = None


def run(inputs, trace=False, **kw):
    global _NC_CACHE
    if _NC_CACHE is None:
        _NC_CACHE = build_module()
    nc = _NC_CACHE
    in_maps = prep_core_inputs(inputs)
    res = run_bass_kernel_spmd(
        nc, in_maps, core_ids=list(range(N_CORES)), trace=trace, **kw
    )
    parts = [r["outp"] for r in res.results]
    return combine_outputs(inputs, parts), res


def kernel(**inputs):
    out, _ = run(inputs)
    return out



# revision 15
# speedup vs baseline: 1.2056x; 1.2056x over previous
"""GAU (gated attention unit) Trainium2 Bass kernel, v2.

Sharding: 8 cores = 4 batches x 2 E-halves.
  core c -> batch b = c//2, E-half h = c%2 (cols h*768:(h+1)*768 of E=1536).

Structural wins over v1 (~272us):
- Kept-key compaction: ~50% of keys are masked out (k=0 -> a=0 exactly);
  the host gathers surviving key tokens (padded to NK*128 with zeroed
  gamma_k/beta_k) so the qk / attn / v matmuls run only on real keys.
- LayerNorm centering is folded into the weights on the host
  (W <- W - colmean(W), exact: (x-mu)@W = x@(W - colmean)), killing the
  rank-2 PSUM correction matmuls and the mu machinery entirely.
- rstd applications: q/k affines and u evac multiply by a
  partition-broadcast rstd row (DVE 32x32 block transpose + block DMAs
  to DRAM in token order + stride-0 broadcast read back); v evac uses an
  ACT copy-with-scale (token-major). u's rstd cancels at the out evac
  (out = rstd * psum, but rstd rides in u), so out evac is a plain copy.
- Single fused pass: prologue computes k/v on compacted keys, then per
  512-query block: qk -> u -> attn -> q(next) -> out with a dense PE
  queue (no phase barrier). Full-T LN stat chains for token chunks 4..15
  run on the DVE inside the main loop where it has slack; their rstd row
  segments are only consumed by the q affines of later blocks.

All matmuls use fp16 operands with fp32 PSUM accumulation. Measured
rel err vs the fp32 reference ~7e-4 (CPU-simulated 7.8e-4).
"""

import numpy as np
from contextlib import ExitStack

import concourse.bass as bass
import concourse.tile as tile
from concourse import bacc, mybir
from concourse.bass_utils import run_bass_kernel_spmd

# Problem dims (hardcoded per the task contract)
B, T, D, S, E = 4, 2048, 768, 128, 1536
EH = E // 2          # per-core E half
P = 128
ND = D // P          # 6 d-chunks
NT = T // P          # 16 token chunks
TT = 512             # query block size
NTT = T // TT        # 4
LN_EPS = 1e-5

F32 = mybir.dt.float32
FP16 = mybir.dt.float16
AF = mybir.ActivationFunctionType
ALU = mybir.AluOpType
NPFP16 = np.float16

N_CORES = 8


def _segs(total, step):
    out = []
    o = 0
    while o < total:
        out.append((o, min(step, total - o)))
        o += step
    return out


def build_module(NK, with_bu=False, with_bv=False):
    TK = NK * P          # padded kept-key count
    nc = bacc.Bacc("TRN2", debug=False, num_devices=N_CORES, num_swdge_queues=4)

    # ---- DRAM I/O ----
    xT_d = nc.dram_tensor("xT", [D, T], FP16, kind="ExternalInput").ap()
    x_d = nc.dram_tensor("x", [T, D], FP16, kind="ExternalInput").ap()
    xkT_d = nc.dram_tensor("xkT", [D, TK], FP16, kind="ExternalInput").ap()
    xk_d = nc.dram_tensor("xk", [TK, D], FP16, kind="ExternalInput").ap()
    gq_d = nc.dram_tensor("gqT", [S, T], FP16, kind="ExternalInput").ap()
    bq_d = nc.dram_tensor("bqT", [S, T], FP16, kind="ExternalInput").ap()
    gk_d = nc.dram_tensor("gkT", [S, TK], FP16, kind="ExternalInput").ap()
    bk_d = nc.dram_tensor("bkT", [S, TK], FP16, kind="ExternalInput").ap()
    wz_d = nc.dram_tensor("Wz", [D, S], FP16, kind="ExternalInput").ap()
    wu_d = nc.dram_tensor("Wu", [D, EH], FP16, kind="ExternalInput").ap()
    wv_d = nc.dram_tensor("Wv", [D, EH], FP16, kind="ExternalInput").ap()
    wo_d = nc.dram_tensor("Wo", [EH, D], FP16, kind="ExternalInput").ap()
    out_d = nc.dram_tensor("outp", [T, D], FP16, kind="ExternalOutput").ap()
    # scratch for the rstd partition-broadcast bounces (linear token order)
    scr_d = nc.dram_tensor("rstd_scr", [NT, P], F32, kind="Internal").ap()
    scrk_d = nc.dram_tensor("rstdk_scr", [NK, P], F32, kind="Internal").ap()
    if with_bu:
        bu_d = nc.dram_tensor("bu", [EH], F32, kind="ExternalInput").ap()
    if with_bv:
        bv_d = nc.dram_tensor("bv", [1, EH], FP16, kind="ExternalInput").ap()

    xT_r = xT_d.rearrange("(c p) t -> p c t", p=P)     # [128, 6, T]
    xkT_r = xkT_d.rearrange("(c p) t -> p c t", p=P)   # [128, 6, TK]
    wu_r = wu_d.rearrange("(c p) e -> p c e", p=P)
    wv_r = wv_d.rearrange("(c p) e -> p c e", p=P)
    wz_r = wz_d.rearrange("(c p) s -> p c s", p=P)
    wo_r = wo_d.rearrange("(c p) d -> p c d", p=P)

    with tile.TileContext(nc) as tc, ExitStack() as ctx:
        # ---------- persistent pools ----------
        persist = ctx.enter_context(tc.tile_pool(name="persist", bufs=1))
        eps_t = persist.tile([P, 1], F32)
        nc.vector.memset(eps_t, LN_EPS)
        warm = persist.tile([P, 1], F32)
        nc.scalar.activation(out=warm, in_=eps_t, func=AF.Sqrt)

        rstd_col = persist.tile([P, 32], F32)     # token-major full-T rstd
        rstd_kcol = persist.tile([P, 32], F32)    # token-major kept rstd
        nc.vector.memset(rstd_col, 0.0)
        nc.vector.memset(rstd_kcol, 0.0)
        rstd_b = persist.tile([P, T], F32)        # rstd row bcast to 128 parts
        rkb = persist.tile([P, TK], F32)          # kept rstd row bcast
        kT = persist.tile([S, TK], FP16)
        v_t = persist.tile([P, NK, EH], FP16)
        wz_t = persist.tile([P, ND, S], FP16)
        wu_t = persist.tile([P, ND, EH], FP16)
        wo_t = persist.tile([P, ND, D], FP16)
        gq_t = persist.tile([S, T], FP16)
        bq_t = persist.tile([S, T], FP16)
        if with_bu:
            bu_t = persist.tile([P, ND], F32)
            nc.gpsimd.dma_start(
                out=bu_t, in_=bu_d.rearrange("(c p) -> p c", p=P))
        if with_bv:
            bvb = persist.tile([P, EH], FP16)
            nc.gpsimd.dma_start(out=bvb, in_=bass.AP(
                tensor=bv_d.tensor, offset=bv_d.offset, ap=[[0, P], [1, EH]]))

        # shared streaming pools (top level: used in prologue and main loop)
        mp = ctx.enter_context(tc.tile_pool(name="mainp", bufs=1))
        sw = ctx.enter_context(tc.tile_pool(name="statp", bufs=1))

        def stats_chain(src, i, col_out):
            """DVE bn_stats chain: src chunk i -> col_out[:, i] = rstd."""
            xt = sw.tile([P, D], FP16, tag="xst", bufs=4)
            nc.scalar.dma_start(out=xt, in_=src[i * P:(i + 1) * P, :])
            st = sw.tile([P, 3, 6], F32, tag="bnst", bufs=2)
            for g in range(3):
                nc.vector.bn_stats(
                    out=st[:, g, :], in_=xt[:, g * 256:(g + 1) * 256])
            mv = sw.tile([P, 2], F32, tag="mv", bufs=2)
            nc.vector.bn_aggr(out=mv, in_=st)
            sd = sw.tile([P, 1], F32, tag="sd", bufs=2)
            nc.scalar.activation(
                out=sd, in_=mv[:, 1:2], func=AF.Sqrt, bias=eps_t, scale=1.0)
            nc.vector.reciprocal(out=col_out[:, i:i + 1], in_=sd)

        def row_bounce(col, scr, rows, b_out, b_off, b_len):
            """col [128, 32] (cols `rows` valid) -> DVE 32x32 block
            transpose -> DRAM rows in token order -> partition-bcast read.
            After the transpose, tr[32*pb + it, r] = col[32*pb + r, it], so
            DRAM row it (token chunk it) gathers [pb, r] slices."""
            r0, rn = rows
            tr = sw.tile([P, 32], F32, tag="tr", bufs=2)
            nc.vector.transpose(out=tr, in_=col)
            for pb in range(4):
                nc.sync.dma_start(
                    out=scr[r0:r0 + rn, 32 * pb:32 * pb + 32],
                    in_=tr[32 * pb + r0:32 * pb + r0 + rn, :])
            nc.sync.dma_start(out=b_out[:, b_off:b_off + b_len], in_=bass.AP(
                tensor=scr.tensor, offset=scr.offset + r0 * P,
                ap=[[0, P], [1, b_len]]))

        # ---------- prologue: stats + k/v on compacted keys ----------
        with (
            tc.tile_pool(name="pw", bufs=1) as pw,
            tc.tile_pool(name="pp", bufs=1, space="PSUM") as pp,
        ):
            # weight/param loads
            nc.gpsimd.dma_start(out=wz_t, in_=wz_r)
            xkT_t = pw.tile([P, ND, TK], FP16)
            for c in range(ND):
                nc.sync.dma_start(out=xkT_t[:, c, :], in_=xkT_r[:, c, :])
            wv_t = pw.tile([P, ND, EH], FP16)
            for c in range(ND):
                nc.gpsimd.dma_start(out=wv_t[:, c, :], in_=wv_r[:, c, :])
            gk_t = pw.tile([S, TK], FP16)
            nc.gpsimd.dma_start(out=gk_t, in_=gk_d)
            bk_t = pw.tile([S, TK], FP16)
            nc.gpsimd.dma_start(out=bk_t, in_=bk_d)
            # first main-loop x tile, early so q(tb0) isn't DMA-bound
            xb0 = mp.tile([P, ND, TT], FP16, tag="xb", bufs=2)
            for c in range(ND):
                nc.sync.dma_start(out=xb0[:, c, :], in_=xT_r[:, c, 0:TT])
            for c in range(ND):
                nc.gpsimd.dma_start(out=wu_t[:, c, :], in_=wu_r[:, c, :])
            nc.gpsimd.dma_start(out=wo_t, in_=wo_r)
            nc.gpsimd.dma_start(out=gq_t, in_=gq_d)
            nc.gpsimd.dma_start(out=bq_t, in_=bq_d)

            # PE: z on compacted keys (held in PSUM until the k affine).
            zsegs = _segs(TK, TT)
            assert len(zsegs) <= 3, "zk PSUM ring supports NK <= 12"
            zk_ps = []
            for (s0, sl) in zsegs:
                zp = pp.tile([S, TT], F32, tag="zk", bufs=3)
                for c in range(ND):
                    nc.tensor.matmul(
                        zp[:, :sl], wz_t[:, c, :], xkT_t[:, c, s0:s0 + sl],
                        start=(c == 0), stop=(c == ND - 1))
                zk_ps.append((zp, s0, sl))

            # PE: v on compacted keys, interleaved with the stats chains so
            # each engine's FIFO sees work in dependency-ready order.
            vgrp = [(ch, e0, ew) for ch in range(NK)
                    for (e0, ew) in ((0, 384), (384, 384))]
            n_inter = len(vgrp)
            if n_inter < NK + 2:       # tiny-NK fallback: emit upfront
                for i in range(NK):
                    stats_chain(xk_d, i, rstd_kcol)
                for i in range(4):
                    stats_chain(x_d, i, rstd_col)
                row_bounce(rstd_col, scr_d, (0, 4), rstd_b, 0, TT)
                row_bounce(rstd_kcol, scrk_d, (0, NK), rkb, 0, TK)
            for i, (ch, e0, ew) in enumerate(vgrp):
                if n_inter >= NK + 2:
                    if i < NK:
                        stats_chain(xk_d, i, rstd_kcol)
                    if i < 4:
                        stats_chain(x_d, i, rstd_col)
                    if i == 4:
                        row_bounce(rstd_col, scr_d, (0, 4), rstd_b, 0, TT)
                    if i == NK + 1:
                        row_bounce(rstd_kcol, scrk_d, (0, NK), rkb, 0, TK)
                vp = pp.tile([P, 384], F32, tag="vm", bufs=3)
                for c in range(ND):
                    nc.tensor.matmul(
                        vp, xkT_t[:, c, ch * P:(ch + 1) * P],
                        wv_t[:, c, e0:e0 + ew],
                        start=(c == 0), stop=(c == ND - 1))
                nc.scalar.activation(
                    out=v_t[:, ch, e0:e0 + ew], in_=vp, func=AF.Copy,
                    scale=rstd_kcol[:, ch:ch + 1])
                if with_bv:
                    nc.vector.tensor_add(
                        out=v_t[:, ch, e0:e0 + ew],
                        in0=v_t[:, ch, e0:e0 + ew], in1=bvb[:, e0:e0 + ew])

            # k affine: k = (zk * rstd) * gamma_k + beta_k  (row-bcast rstd)
            for (zp, s0, sl) in zk_ps:
                zs = sw.tile([S, TT], F32, tag="zsk", bufs=2)
                nc.vector.tensor_mul(
                    out=zs[:, :sl], in0=zp[:, :sl], in1=rkb[:S, s0:s0 + sl])
                kf = sw.tile([S, TT], F32, tag="kfk", bufs=2)
                nc.vector.tensor_mul(
                    out=kf[:, :sl], in0=zs[:, :sl], in1=gk_t[:, s0:s0 + sl])
                nc.vector.tensor_add(
                    out=kT[:, s0:s0 + sl], in0=kf[:, :sl],
                    in1=bk_t[:, s0:s0 + sl])

            # q(tb0) matmuls at prologue end (affine drains into main loop)
            zq0 = pp.tile([S, TT], F32, tag="q0", bufs=1)
            for c in range(ND):
                nc.tensor.matmul(
                    zq0, wz_t[:, c, :], xb0[:, c, :],
                    start=(c == 0), stop=(c == ND - 1))
            zqs = sw.tile([S, TT], F32, tag="zq0", bufs=1)
            nc.vector.tensor_mul(out=zqs, in0=zq0, in1=rstd_b[:S, 0:TT])
            qf0 = sw.tile([S, TT], F32, tag="qf0", bufs=1)
            nc.vector.tensor_mul(out=qf0, in0=zqs, in1=gq_t[:, 0:TT])
            qT_cur = mp.tile([S, TT], FP16, tag="qT", bufs=2)
            nc.vector.tensor_add(out=qT_cur, in0=qf0, in1=bq_t[:, 0:TT])

        # ---------- fused main loop over 512-query blocks ----------
        # Full-T stat chains 4..15 + rstd_b segs 1..3 are sprinkled in:
        # seg s is consumed by the q affine of block s, emitted in
        # iteration s-1, so chains 4s..4s+3 must be emitted before that.
        def late_stats(tb, phase):
            if tb <= 2 and phase == 0:
                for i in (4 * tb + 4, 4 * tb + 5):
                    if i < NT:
                        stats_chain(x_d, i, rstd_col)
            if tb <= 2 and phase == 1:
                for i in (4 * tb + 6, 4 * tb + 7):
                    if i < NT:
                        stats_chain(x_d, i, rstd_col)
                s = tb + 1
                row_bounce(rstd_col, scr_d, (4 * s, 4), rstd_b,
                           s * TT, TT)

        with (
            tc.tile_pool(name="mw", bufs=1) as mw,
            tc.tile_pool(name="ps", bufs=1, space="PSUM") as ps,
        ):
            xb_cur = xb0
            for tb in range(NTT):
                ts_ = slice(tb * TT, (tb + 1) * TT)
                if tb < NTT - 1:
                    xb_next = mp.tile([P, ND, TT], FP16, tag="xb", bufs=2)
                    for c in range(ND):
                        nc.sync.dma_start(
                            out=xb_next[:, c, :],
                            in_=xT_r[:, c, (tb + 1) * TT:(tb + 2) * TT])

                # --- qk + a = relu(qk)^2 (DVE relu into fp16, DVE square) ---
                aTt = mw.tile([P, NK, TT], FP16, tag="aT", bufs=2)
                for uc in range(NK):
                    qkp = ps.tile([P, TT], F32, tag="qk", bufs=3)
                    nc.tensor.matmul(
                        qkp, kT[:, uc * P:(uc + 1) * P], qT_cur,
                        start=True, stop=True)
                    rt = mw.tile([P, TT], FP16, tag="rt", bufs=3)
                    nc.vector.tensor_scalar_max(rt, qkp, 0.0)
                    nc.vector.tensor_mul(out=aTt[:, uc, :], in0=rt, in1=rt)
                late_stats(tb, 0)

                # --- u (E-major; rstd applied with the row broadcast) ---
                uTt = mw.tile([P, ND, TT], FP16, tag="uT", bufs=2)
                for e in range(ND):
                    up = ps.tile([P, TT], F32, tag="mm", bufs=3)
                    for c in range(ND):
                        nc.tensor.matmul(
                            up, wu_t[:, c, e * P:(e + 1) * P], xb_cur[:, c, :],
                            start=(c == 0), stop=(c == ND - 1))
                    if with_bu:
                        uf = mw.tile([P, TT], F32, tag="uf", bufs=2)
                        nc.vector.tensor_mul(
                            out=uf, in0=up, in1=rstd_b[:, ts_])
                        nc.scalar.activation(
                            out=uTt[:, e, :], in_=uf, func=AF.Identity,
                            bias=bu_t[:, e:e + 1])
                    else:
                        nc.vector.tensor_mul(
                            out=uTt[:, e, :], in0=up, in1=rstd_b[:, ts_])
                late_stats(tb, 1)

                # --- attn = v^T @ a^T, then g = u * attn in place ---
                for e in range(ND):
                    ap_ = ps.tile([P, TT], F32, tag="at", bufs=2)
                    for uc in range(NK):
                        nc.tensor.matmul(
                            ap_, v_t[:, uc, e * P:(e + 1) * P], aTt[:, uc, :],
                            start=(uc == 0), stop=(uc == NK - 1))
                    nc.vector.tensor_mul(
                        out=uTt[:, e, :], in0=ap_, in1=uTt[:, e, :])

                # --- q for the next block ---
                if tb < NTT - 1:
                    nts = slice((tb + 1) * TT, (tb + 2) * TT)
                    zp = ps.tile([S, TT], F32, tag="mm", bufs=3)
                    for c in range(ND):
                        nc.tensor.matmul(
                            zp, wz_t[:, c, :], xb_next[:, c, :],
                            start=(c == 0), stop=(c == ND - 1))
                    zs = mw.tile([S, TT], F32, tag="zs", bufs=2)
                    nc.vector.tensor_mul(out=zs, in0=zp, in1=rstd_b[:S, nts])
                    qf = mw.tile([S, TT], F32, tag="qf", bufs=2)
                    nc.vector.tensor_mul(out=qf, in0=zs, in1=gq_t[:, nts])
                    qT_next = mp.tile([S, TT], FP16, tag="qT", bufs=2)
                    nc.vector.tensor_add(out=qT_next, in0=qf, in1=bq_t[:, nts])

                # --- out = g^T @ Wo (plain copy; rstd rides in u) ---
                for tch in range(TT // P):
                    it = tb * (TT // P) + tch
                    tc_ = slice(it * P, (it + 1) * P)
                    osb = mw.tile([P, D], FP16, tag="osb", bufs=3)
                    for (d0, dw) in ((0, 384), (384, 384)):
                        op_ = ps.tile([P, TT], F32, tag="mm", bufs=3)
                        for e in range(ND):
                            nc.tensor.matmul(
                                op_[:, :dw], uTt[:, e, tch * P:(tch + 1) * P],
                                wo_t[:, e, d0:d0 + dw],
                                start=(e == 0), stop=(e == ND - 1))
                        nc.scalar.copy(out=osb[:, d0:d0 + dw], in_=op_[:, :dw])
                    nc.sync.dma_start(out=out_d[tc_, :], in_=osb)

                if tb < NTT - 1:
                    xb_cur = xb_next
                    qT_cur = qT_next

    nc.finalize()
    return nc


def prep_core_inputs(inputs):
    """Host-side prep: fold LN centering/scale into weights, fold bz into
    the q/k affines, gather kept (unmasked) key tokens, slice E halves."""
    f = np.float32
    x = np.asarray(inputs["x"], f)
    mask = np.asarray(inputs["mask"])
    ln_w = np.asarray(inputs["ln_w"], f)
    ln_b = np.asarray(inputs["ln_b"], f)
    Wz = np.asarray(inputs["Wz"], f)
    bz = np.asarray(inputs["bz"], f)
    Wu = np.asarray(inputs["Wu"], f)
    bu = np.asarray(inputs["bu"], f)
    Wv = np.asarray(inputs["Wv"], f)
    bv = np.asarray(inputs["bv"], f)
    Wo = np.asarray(inputs["Wo"], f)
    gq = np.asarray(inputs["gamma_q"], f)
    bq = np.asarray(inputs["beta_q"], f)
    gk = np.asarray(inputs["gamma_k"], f)
    bk = np.asarray(inputs["beta_k"], f)

    # fold ln_w into weights, then fold the LN centering projection:
    # ((x - mu) * rstd) @ W = rstd * (x @ (W - colmean(W)))
    Wz_e = ln_w[:, None] * Wz
    Wu_e = ln_w[:, None] * Wu
    Wv_e = ln_w[:, None] * Wv
    Wz_c = (Wz_e - Wz_e.mean(0, keepdims=True)).astype(NPFP16)
    Wu_c = (Wu_e - Wu_e.mean(0, keepdims=True)).astype(NPFP16)
    Wv_c = (Wv_e - Wv_e.mean(0, keepdims=True)).astype(NPFP16)
    # biases: z-path bias folds exactly into the affines
    bz_e = ln_b @ Wz + bz
    bu_e = ln_b @ Wu + bu
    bv_e = ln_b @ Wv + bv
    with_bu = bool(np.any(bu_e != 0))
    with_bv = bool(np.any(bv_e != 0))

    bq_f = bq + bz_e[None, :] * gq      # [T, S]
    bk_f = bk + bz_e[None, :] * gk

    keeps = [np.where(~mask[b])[0] for b in range(B)]
    NK = max(1, -(-max(len(kk) for kk in keeps) // P))
    TK = NK * P

    gqT = np.ascontiguousarray(gq.T.astype(NPFP16))
    bqT = np.ascontiguousarray(bq_f.T.astype(NPFP16))

    in_maps = []
    for c in range(N_CORES):
        b, h = c // 2, c % 2
        cols = slice(h * EH, (h + 1) * EH)
        kidx = keeps[b]
        nk = len(kidx)
        xk = np.zeros((TK, D), NPFP16)
        xk[:nk] = x[b][kidx].astype(NPFP16)
        gkT = np.zeros((S, TK), NPFP16)
        gkT[:, :nk] = gk[kidx].T.astype(NPFP16)
        bkT = np.zeros((S, TK), NPFP16)
        bkT[:, :nk] = bk_f[kidx].T.astype(NPFP16)
        m = {
            "x": np.ascontiguousarray(x[b].astype(NPFP16)),
            "xT": np.ascontiguousarray(x[b].T.astype(NPFP16)),
            "xk": xk,
            "xkT": np.ascontiguousarray(xk.T),
            "gqT": gqT,
            "bqT": bqT,
            "gkT": gkT,
            "bkT": bkT,
            "Wz": np.ascontiguousarray(Wz_c),
            "Wu": np.ascontiguousarray(Wu_c[:, cols]),
            "Wv": np.ascontiguousarray(Wv_c[:, cols]),
            "Wo": np.ascontiguousarray(Wo[cols, :].astype(NPFP16)),
        }
        if with_bu:
            m["bu"] = np.ascontiguousarray(bu_e[cols])
        if with_bv:
            m["bv"] = np.ascontiguousarray(
                bv_e[cols].reshape(1, EH).astype(NPFP16))
        in_maps.append(m)
    return in_maps, NK, with_bu, with_bv


def combine_outputs(inputs, parts):
    """parts: list of 8 [T, D] fp16 partials -> full [B, T, D] fp32."""
    f = np.float32
    x = np.asarray(inputs["x"], f)
    bo = np.asarray(inputs["bo"], f)
    out = np.empty((B, T, D), f)
    for b in range(B):
        out[b] = (parts[2 * b].astype(f) + parts[2 * b + 1].astype(f)
                  + bo[None, :] + x[b])
    return out


_NC_CACHE = {}


def run(inputs, trace=False, **kw):
    in_maps, NK, with_bu, with_bv = prep_core_inputs(inputs)
    key = (NK, with_bu, with_bv)
    if key not in _NC_CACHE:
        _NC_CACHE[key] = build_module(NK, with_bu, with_bv)
    nc = _NC_CACHE[key]
    res = run_bass_kernel_spmd(
        nc, in_maps, core_ids=list(range(N_CORES)), trace=trace, **kw
    )
    parts = [r["outp"] for r in res.results]
    return combine_outputs(inputs, parts), res


def kernel(**inputs):
    out, _ = run(inputs)
    return out


# revision 17
# speedup vs baseline: 1.3523x; 1.1217x over previous
"""GAU (gated attention unit) Trainium2 Bass kernel, v3.

Sharding: 8 cores = 4 batches x 2 E-halves.
  core c -> batch b = c//2, E-half h = c%2 (cols h*768:(h+1)*768 of E=1536).

v1 (~272us) -> v2 (~227us): kept-key compaction (half the keys are
masked out; gather survivors, padded to NK*128), LN centering folded
into the weights on host (exact), fused single pass.

v2 -> v3: engine rebalance so PSUM evacuation never stalls the PE.
- sqrt(rstd) folds: q'' = q_true * sqrt(rstd_q) per query column and
  k'' = k_true * sqrt(rstd_k) per key column. relu^2 is homogeneous of
  degree 2, so a'' = rstd_q[qt] * rstd_k[kt] * a_true. The rstd_k factor
  is exactly v's LN scale (v evac becomes a plain copy), and the rstd_q
  factor is exactly u's LN scale, which distributes over the final
  contraction (u and out evacs become plain copies). All rstd work now
  lives in the q/k affines as row broadcasts of rstd^1.5 and rstd^0.5:
    q'' = zq_raw * (rstd^1.5 * gamma_q) + beta_q * rstd^0.5
- LN stat chains: DVE bn_stats -> batched sqrt/recip finalizers; the 12
  full-T chains for query blocks 1..3 run inside the main loop where the
  DVE has slack. Rows are built with DVE 32x32 block transposes + block
  DMAs to DRAM in token order + stride-0 broadcast reads.
- relu alternates DVE/ACT; squares on DVE in fp16 (2x); k affine runs
  mostly in fp16.

All matmuls use fp16 operands with fp32 PSUM accumulation. Measured
rel err vs the fp32 reference ~9e-4.
"""

import numpy as np
from contextlib import ExitStack

import concourse.bass as bass
import concourse.tile as tile
from concourse import bacc, mybir
from concourse.bass_utils import run_bass_kernel_spmd

# Problem dims (hardcoded per the task contract)
B, T, D, S, E = 4, 2048, 768, 128, 1536
EH = E // 2          # per-core E half
P = 128
ND = D // P          # 6 d-chunks
NT = T // P          # 16 token chunks
TT = 512             # query block size
NTT = T // TT        # 4
LN_EPS = 1e-5

F32 = mybir.dt.float32
FP16 = mybir.dt.float16
AF = mybir.ActivationFunctionType
ALU = mybir.AluOpType
NPFP16 = np.float16

N_CORES = 8


def _segs(total, step):
    out = []
    o = 0
    while o < total:
        out.append((o, min(step, total - o)))
        o += step
    return out


def build_module(NK, with_bu=False, with_bv=False):
    TK = NK * P          # padded kept-key count
    nc = bacc.Bacc("TRN2", debug=False, num_devices=N_CORES, num_swdge_queues=4)

    # ---- DRAM I/O ----
    xT_d = nc.dram_tensor("xT", [D, T], FP16, kind="ExternalInput").ap()
    x_d = nc.dram_tensor("x", [T, D], FP16, kind="ExternalInput").ap()
    xkT_d = nc.dram_tensor("xkT", [D, TK], FP16, kind="ExternalInput").ap()
    xk_d = nc.dram_tensor("xk", [TK, D], FP16, kind="ExternalInput").ap()
    gq_d = nc.dram_tensor("gqT", [S, T], FP16, kind="ExternalInput").ap()
    bq_d = nc.dram_tensor("bqT", [S, T], FP16, kind="ExternalInput").ap()
    gk_d = nc.dram_tensor("gkT", [S, TK], FP16, kind="ExternalInput").ap()
    bk_d = nc.dram_tensor("bkT", [S, TK], FP16, kind="ExternalInput").ap()
    wz_d = nc.dram_tensor("Wz", [D, S], FP16, kind="ExternalInput").ap()
    wu_d = nc.dram_tensor("Wu", [D, EH], FP16, kind="ExternalInput").ap()
    wv_d = nc.dram_tensor("Wv", [D, EH], FP16, kind="ExternalInput").ap()
    wo_d = nc.dram_tensor("Wo", [EH, D], FP16, kind="ExternalInput").ap()
    out_d = nc.dram_tensor("outp", [T, D], FP16, kind="ExternalOutput").ap()
    # scratch rows for the rstd^1.5 / rstd^0.5 bounces (token order)
    s15_d = nc.dram_tensor("r15_scr", [NT, P], F32, kind="Internal").ap()
    s05_d = nc.dram_tensor("r05_scr", [NT, P], F32, kind="Internal").ap()
    s15k_d = nc.dram_tensor("r15k_scr", [NK, P], F32, kind="Internal").ap()
    s05k_d = nc.dram_tensor("r05k_scr", [NK, P], F32, kind="Internal").ap()
    if with_bu:
        bu_d = nc.dram_tensor("bu", [EH], F32, kind="ExternalInput").ap()
    if with_bv:
        bv_d = nc.dram_tensor("bv", [1, EH], FP16, kind="ExternalInput").ap()

    xT_r = xT_d.rearrange("(c p) t -> p c t", p=P)     # [128, 6, T]
    xkT_r = xkT_d.rearrange("(c p) t -> p c t", p=P)   # [128, 6, TK]
    wu_r = wu_d.rearrange("(c p) e -> p c e", p=P)
    wv_r = wv_d.rearrange("(c p) e -> p c e", p=P)
    wz_r = wz_d.rearrange("(c p) s -> p c s", p=P)
    wo_r = wo_d.rearrange("(c p) d -> p c d", p=P)

    with tile.TileContext(nc) as tc, ExitStack() as ctx:
        # ---------- persistent pools ----------
        persist = ctx.enter_context(tc.tile_pool(name="persist", bufs=1))
        eps_t = persist.tile([P, 1], F32)
        nc.vector.memset(eps_t, LN_EPS)
        warm = persist.tile([P, 1], F32)
        nc.scalar.activation(out=warm, in_=eps_t, func=AF.Sqrt)

        r15_col = persist.tile([P, 32], F32)   # token-major rstd^1.5, full T
        r05_col = persist.tile([P, 32], F32)
        r15k_col = persist.tile([P, 32], F32)  # token-major, kept tokens
        r05k_col = persist.tile([P, 32], F32)
        for t_ in (r15_col, r05_col, r15k_col, r05k_col):
            nc.vector.memset(t_, 0.0)
        mvs_f = persist.tile([P, NT, 2], F32)  # bn_aggr (mean, var) full T
        mvs_k = persist.tile([P, 32, 2], F32)  # bn_aggr (mean, var) kept
        r15b = persist.tile([P, T], F32)       # rstd^1.5 row, all partitions
        r05b = persist.tile([P, T], F32)
        r15kb = persist.tile([P, TK], F32)
        r05kb = persist.tile([P, TK], F32)
        bq2 = persist.tile([S, T], FP16)       # beta_q * rstd^0.5
        bk2 = persist.tile([S, TK], FP16)
        kT = persist.tile([S, TK], FP16)
        v_t = persist.tile([P, NK, EH], FP16)
        wz_t = persist.tile([P, ND, S], FP16)
        wu_t = persist.tile([P, ND, EH], FP16)
        wo_t = persist.tile([P, ND, D], FP16)
        gq_t = persist.tile([S, T], FP16)
        bq_t = persist.tile([S, T], FP16)
        if with_bu:
            bu_t = persist.tile([P, ND], F32)
            nc.gpsimd.dma_start(
                out=bu_t, in_=bu_d.rearrange("(c p) -> p c", p=P))
            rstd_b = persist.tile([P, T], F32)
        if with_bv:
            bvb = persist.tile([P, EH], FP16)
            nc.gpsimd.dma_start(out=bvb, in_=bass.AP(
                tensor=bv_d.tensor, offset=bv_d.offset, ap=[[0, P], [1, EH]]))
            rstd_kcol = persist.tile([P, 32], F32)
            rkb = persist.tile([P, TK], F32)

        # shared streaming pools (top level: used in prologue and main loop)
        mp = ctx.enter_context(tc.tile_pool(name="mainp", bufs=1))
        sw = ctx.enter_context(tc.tile_pool(name="statp", bufs=1))

        def stats_chain(src, i, mvs):
            """DVE bn_stats chain: src chunk i -> mvs[:, i, :] = (mean, var)."""
            xt = sw.tile([P, D], FP16, tag="xst", bufs=4)
            nc.scalar.dma_start(out=xt, in_=src[i * P:(i + 1) * P, :])
            st = sw.tile([P, 3, 6], F32, tag="bnst", bufs=2)
            for g in range(3):
                nc.vector.bn_stats(
                    out=st[:, g, :], in_=xt[:, g * 256:(g + 1) * 256])
            nc.vector.bn_aggr(out=mvs[:, i, :], in_=st)

        def batch_rstd(mvs, cols, c15, c05, extra_col=None):
            """(mean,var) cols -> rstd^1.5 / rstd^0.5 columns (batched)."""
            c0, cn = cols
            sd = sw.tile([P, 16], F32, tag="sd", bufs=2)
            nc.scalar.activation(
                out=sd[:, :cn], in_=mvs[:, c0:c0 + cn, 1], func=AF.Sqrt,
                bias=eps_t, scale=1.0)
            rst = sw.tile([P, 16], F32, tag="rst", bufs=2)
            nc.vector.reciprocal(out=rst[:, :cn], in_=sd[:, :cn])
            nc.scalar.activation(
                out=c05[:, c0:c0 + cn], in_=rst[:, :cn], func=AF.Sqrt)
            nc.vector.tensor_mul(
                out=c15[:, c0:c0 + cn], in0=rst[:, :cn],
                in1=c05[:, c0:c0 + cn])
            if extra_col is not None:
                nc.vector.tensor_mul(
                    out=extra_col[:, c0:c0 + cn], in0=c05[:, c0:c0 + cn],
                    in1=c05[:, c0:c0 + cn])

        def row_bounce(col, scr, rows, b_out, b_off, b_len):
            """col [128, 32] -> DVE 32x32 block transpose -> DRAM rows in
            token order -> stride-0 partition-broadcast read into b_out."""
            r0, rn = rows
            tr = sw.tile([P, 32], F32, tag="tr", bufs=2)
            nc.vector.transpose(out=tr, in_=col)
            for pb in range(4):
                nc.sync.dma_start(
                    out=scr[r0:r0 + rn, 32 * pb:32 * pb + 32],
                    in_=tr[32 * pb + r0:32 * pb + r0 + rn, :])
            nc.sync.dma_start(out=b_out[:, b_off:b_off + b_len], in_=bass.AP(
                tensor=scr.tensor, offset=scr.offset + r0 * P,
                ap=[[0, P], [1, b_len]]))

        def full_seg_rows(s):
            """rstd^1.5/^0.5 rows + bq2 for full-T segment s."""
            batch_rstd(mvs_f, (4 * s, 4), r15_col, r05_col)
            row_bounce(r15_col, s15_d, (4 * s, 4), r15b, s * TT, TT)
            row_bounce(r05_col, s05_d, (4 * s, 4), r05b, s * TT, TT)
            sl = slice(s * TT, (s + 1) * TT)
            if with_bu:
                # general path: q affine carries plain rstd; u evac scales
                nc.vector.tensor_mul(
                    out=rstd_b[:, sl], in0=r05b[:, sl], in1=r05b[:, sl])
                nc.vector.tensor_copy(out=bq2[:, sl], in_=bq_t[:, sl])
            else:
                nc.vector.tensor_mul(
                    out=bq2[:, sl], in0=bq_t[:, sl], in1=r05b[:S, sl])

        # ---------- prologue: stats + k/v on compacted keys ----------
        with (
            tc.tile_pool(name="pw", bufs=1) as pw,
            tc.tile_pool(name="pp", bufs=1, space="PSUM") as pp,
        ):
            # weight/param loads. gpsimd queue: needed-first order.
            nc.gpsimd.dma_start(out=wz_t, in_=wz_r)
            wv_t = pw.tile([P, ND, EH], FP16)
            for c in range(ND):
                nc.gpsimd.dma_start(out=wv_t[:, c, :], in_=wv_r[:, c, :])
            gk_t = pw.tile([S, TK], FP16)
            nc.gpsimd.dma_start(out=gk_t, in_=gk_d)
            bk_t = pw.tile([S, TK], FP16)
            nc.gpsimd.dma_start(out=bk_t, in_=bk_d)
            for c in range(ND):
                nc.gpsimd.dma_start(out=wu_t[:, c, :], in_=wu_r[:, c, :])
            nc.gpsimd.dma_start(out=wo_t, in_=wo_r)
            # sync queue: x for the z/v matmuls
            xkT_t = pw.tile([P, ND, TK], FP16)
            for c in range(ND):
                nc.sync.dma_start(out=xkT_t[:, c, :], in_=xkT_r[:, c, :])
            xb0 = mp.tile([P, ND, TT], FP16, tag="xb", bufs=2)
            for c in range(ND):
                nc.sync.dma_start(out=xb0[:, c, :], in_=xT_r[:, c, 0:TT])
            # scalar queue: stats streams + q-affine params
            # (stats chains issue their own DMAs on the scalar queue)

            # PE: z on compacted keys (held in PSUM until the k affine).
            zsegs = _segs(TK, TT)
            assert len(zsegs) <= 3, "zk PSUM ring supports NK <= 12"
            zk_ps = []
            for (s0, sl) in zsegs:
                zp = pp.tile([S, TT], F32, tag="zk", bufs=3)
                for c in range(ND):
                    nc.tensor.matmul(
                        zp[:, :sl], wz_t[:, c, :], xkT_t[:, c, s0:s0 + sl],
                        start=(c == 0), stop=(c == ND - 1))
                zk_ps.append((zp, s0, sl))

            # PE: v on compacted keys; kept/seg0 stats interleaved.
            vgrp = [(ch, e0, ew) for ch in range(NK)
                    for (e0, ew) in ((0, 384), (384, 384))]
            n_inter = len(vgrp)

            def pro_stats(i):
                if i < NK:
                    stats_chain(xk_d, i, mvs_k)
                if i < 4:
                    stats_chain(x_d, i, mvs_f)
                if i == NK:
                    nc.gpsimd.dma_start(out=gq_t, in_=gq_d)
                    nc.gpsimd.dma_start(out=bq_t, in_=bq_d)
                if i == NK + 1:
                    batch_rstd(mvs_k, (0, NK), r15k_col, r05k_col,
                               extra_col=rstd_kcol if with_bv else None)
                    row_bounce(r15k_col, s15k_d, (0, NK), r15kb, 0, TK)
                    row_bounce(r05k_col, s05k_d, (0, NK), r05kb, 0, TK)
                    if with_bv:
                        nc.vector.tensor_mul(
                            out=rkb, in0=r05kb, in1=r05kb)
                if i == NK + 2:
                    full_seg_rows(0)

            if n_inter < NK + 3:       # tiny-NK fallback: emit upfront
                for i in range(NK + 3):
                    pro_stats(i)
            for i, (ch, e0, ew) in enumerate(vgrp):
                if n_inter >= NK + 3:
                    pro_stats(i)
                vp = pp.tile([P, 384], F32, tag="vm", bufs=3)
                for c in range(ND):
                    nc.tensor.matmul(
                        vp, xkT_t[:, c, ch * P:(ch + 1) * P],
                        wv_t[:, c, e0:e0 + ew],
                        start=(c == 0), stop=(c == ND - 1))
                if with_bv:
                    nc.scalar.activation(
                        out=v_t[:, ch, e0:e0 + ew], in_=vp, func=AF.Copy,
                        scale=rstd_kcol[:, ch:ch + 1])
                    nc.vector.tensor_add(
                        out=v_t[:, ch, e0:e0 + ew],
                        in0=v_t[:, ch, e0:e0 + ew], in1=bvb[:, e0:e0 + ew])
                else:
                    nc.scalar.copy(out=v_t[:, ch, e0:e0 + ew], in_=vp)

            # k affine. fast path: k'' = zk*(r15kb*gamma) + beta*r05kb
            # (the extra sqrt(rstd_k) cancels against v's missing LN scale
            # through relu^2 homogeneity). general (bv): true affine.
            kgain = rkb if with_bv else r15kb
            if with_bv:
                nc.vector.tensor_copy(out=bk2, in_=bk_t)
            else:
                nc.vector.tensor_mul(out=bk2, in0=bk_t, in1=r05kb[:S, :])
            for (zp, s0, sl) in zk_ps:
                t1 = sw.tile([S, TT], FP16, tag="kt1", bufs=2)
                nc.vector.tensor_mul(
                    out=t1[:, :sl], in0=zp[:, :sl], in1=kgain[:S, s0:s0 + sl])
                t2 = sw.tile([S, TT], FP16, tag="kt2", bufs=2)
                nc.vector.tensor_mul(
                    out=t2[:, :sl], in0=t1[:, :sl], in1=gk_t[:, s0:s0 + sl])
                nc.vector.tensor_add(
                    out=kT[:, s0:s0 + sl], in0=t2[:, :sl],
                    in1=bk2[:, s0:s0 + sl])

            # q(tb0) matmuls at prologue end (affine drains into main loop)
            zq0 = pp.tile([S, TT], F32, tag="q0", bufs=1)
            for c in range(ND):
                nc.tensor.matmul(
                    zq0, wz_t[:, c, :], xb0[:, c, :],
                    start=(c == 0), stop=(c == ND - 1))
            qgain = rstd_b if with_bu else r15b
            t1 = sw.tile([S, TT], FP16, tag="qt1", bufs=2)
            nc.vector.tensor_mul(out=t1, in0=zq0, in1=qgain[:S, 0:TT])
            t2 = sw.tile([S, TT], FP16, tag="qt2", bufs=2)
            nc.vector.tensor_mul(out=t2, in0=t1, in1=gq_t[:, 0:TT])
            qT_cur = mp.tile([S, TT], FP16, tag="qT", bufs=2)
            nc.vector.tensor_add(out=qT_cur, in0=t2, in1=bq2[:, 0:TT])

        # ---------- fused main loop over 512-query blocks ----------
        # Full-T stat chains 4..15 + row segs 1..3 run inside the loop:
        # seg s is consumed by the q affine of block s, emitted in
        # iteration s-1, so chains 4s..4s+3 are emitted just before it.
        def late_stats(tb, phase):
            if tb <= 2 and phase == 0:
                for i in (4 * tb + 4, 4 * tb + 5):
                    if i < NT:
                        stats_chain(x_d, i, mvs_f)
            if tb <= 2 and phase == 1:
                for i in (4 * tb + 6, 4 * tb + 7):
                    if i < NT:
                        stats_chain(x_d, i, mvs_f)
                full_seg_rows(tb + 1)

        with (
            tc.tile_pool(name="mw", bufs=1) as mw,
            tc.tile_pool(name="ps", bufs=1, space="PSUM") as ps,
        ):
            xb_cur = xb0
            for tb in range(NTT):
                ts_ = slice(tb * TT, (tb + 1) * TT)
                if tb < NTT - 1:
                    xb_next = mp.tile([P, ND, TT], FP16, tag="xb", bufs=2)
                    for c in range(ND):
                        nc.sync.dma_start(
                            out=xb_next[:, c, :],
                            in_=xT_r[:, c, (tb + 1) * TT:(tb + 2) * TT])

                # --- qk + a = relu(qk)^2 (relu alternates DVE/ACT) ---
                aTt = mw.tile([P, NK, TT], FP16, tag="aT", bufs=2)
                for uc in range(NK):
                    qkp = ps.tile([P, TT], F32, tag="qk", bufs=3)
                    nc.tensor.matmul(
                        qkp, kT[:, uc * P:(uc + 1) * P], qT_cur,
                        start=True, stop=True)
                    rt = mw.tile([P, TT], FP16, tag="rt", bufs=3)
                    if uc % 3 == 2:
                        nc.scalar.activation(out=rt, in_=qkp, func=AF.Relu)
                    else:
                        nc.vector.tensor_scalar_max(rt, qkp, 0.0)
                    nc.vector.tensor_mul(out=aTt[:, uc, :], in0=rt, in1=rt)
                late_stats(tb, 0)

                # --- u (plain evac: rstd_q rides in via the q affine) ---
                uTt = mw.tile([P, ND, TT], FP16, tag="uT", bufs=2)
                for e in range(ND):
                    up = ps.tile([P, TT], F32, tag="mm", bufs=3)
                    for c in range(ND):
                        nc.tensor.matmul(
                            up, wu_t[:, c, e * P:(e + 1) * P], xb_cur[:, c, :],
                            start=(c == 0), stop=(c == ND - 1))
                    if with_bu:
                        uf = mw.tile([P, TT], F32, tag="uf", bufs=2)
                        nc.vector.tensor_mul(
                            out=uf, in0=up, in1=rstd_b[:, ts_])
                        nc.scalar.activation(
                            out=uTt[:, e, :], in_=uf, func=AF.Identity,
                            bias=bu_t[:, e:e + 1])
                    else:
                        nc.scalar.copy(out=uTt[:, e, :], in_=up)
                late_stats(tb, 1)

                # --- attn = v^T @ a^T, then g = u * attn in place ---
                for e in range(ND):
                    ap_ = ps.tile([P, TT], F32, tag="at", bufs=2)
                    for uc in range(NK):
                        nc.tensor.matmul(
                            ap_, v_t[:, uc, e * P:(e + 1) * P], aTt[:, uc, :],
                            start=(uc == 0), stop=(uc == NK - 1))
                    nc.vector.tensor_mul(
                        out=uTt[:, e, :], in0=ap_, in1=uTt[:, e, :])

                # --- q for the next block ---
                if tb < NTT - 1:
                    nts = slice((tb + 1) * TT, (tb + 2) * TT)
                    zp = ps.tile([S, TT], F32, tag="mm", bufs=3)
                    for c in range(ND):
                        nc.tensor.matmul(
                            zp, wz_t[:, c, :], xb_next[:, c, :],
                            start=(c == 0), stop=(c == ND - 1))
                    qgain = rstd_b if with_bu else r15b
                    t1 = mw.tile([S, TT], FP16, tag="t1", bufs=2)
                    nc.vector.tensor_mul(out=t1, in0=zp, in1=qgain[:S, nts])
                    t2 = mw.tile([S, TT], FP16, tag="t2", bufs=2)
                    nc.vector.tensor_mul(out=t2, in0=t1, in1=gq_t[:, nts])
                    qT_next = mp.tile([S, TT], FP16, tag="qT", bufs=2)
                    nc.vector.tensor_add(
                        out=qT_next, in0=t2, in1=bq2[:, nts])

                # --- out = g^T @ Wo (plain copy; rstd_q rides in u) ---
                for tch in range(TT // P):
                    it = tb * (TT // P) + tch
                    tc_ = slice(it * P, (it + 1) * P)
                    osb = mw.tile([P, D], FP16, tag="osb", bufs=3)
                    for (d0, dw) in ((0, 384), (384, 384)):
                        op_ = ps.tile([P, TT], F32, tag="mm", bufs=3)
                        for e in range(ND):
                            nc.tensor.matmul(
                                op_[:, :dw], uTt[:, e, tch * P:(tch + 1) * P],
                                wo_t[:, e, d0:d0 + dw],
                                start=(e == 0), stop=(e == ND - 1))
                        nc.scalar.copy(out=osb[:, d0:d0 + dw], in_=op_[:, :dw])
                    nc.sync.dma_start(out=out_d[tc_, :], in_=osb)

                if tb < NTT - 1:
                    xb_cur = xb_next
                    qT_cur = qT_next

    nc.finalize()
    return nc


def prep_core_inputs(inputs):
    """Host-side prep: fold LN centering/scale into weights, fold bz into
    the q/k affines, gather kept (unmasked) key tokens, slice E halves."""
    f = np.float32
    x = np.asarray(inputs["x"], f)
    mask = np.asarray(inputs["mask"])
    ln_w = np.asarray(inputs["ln_w"], f)
    ln_b = np.asarray(inputs["ln_b"], f)
    Wz = np.asarray(inputs["Wz"], f)
    bz = np.asarray(inputs["bz"], f)
    Wu = np.asarray(inputs["Wu"], f)
    bu = np.asarray(inputs["bu"], f)
    Wv = np.asarray(inputs["Wv"], f)
    bv = np.asarray(inputs["bv"], f)
    Wo = np.asarray(inputs["Wo"], f)
    gq = np.asarray(inputs["gamma_q"], f)
    bq = np.asarray(inputs["beta_q"], f)
    gk = np.asarray(inputs["gamma_k"], f)
    bk = np.asarray(inputs["beta_k"], f)

    # fold ln_w into weights, then fold the LN centering projection:
    # ((x - mu) * rstd) @ W = rstd * (x @ (W - colmean(W)))
    Wz_e = ln_w[:, None] * Wz
    Wu_e = ln_w[:, None] * Wu
    Wv_e = ln_w[:, None] * Wv
    Wz_c = (Wz_e - Wz_e.mean(0, keepdims=True)).astype(NPFP16)
    Wu_c = (Wu_e - Wu_e.mean(0, keepdims=True)).astype(NPFP16)
    Wv_c = (Wv_e - Wv_e.mean(0, keepdims=True)).astype(NPFP16)
    # biases: z-path bias folds exactly into the affines
    bz_e = ln_b @ Wz + bz
    bu_e = ln_b @ Wu + bu
    bv_e = ln_b @ Wv + bv
    with_bu = bool(np.any(bu_e != 0))
    with_bv = bool(np.any(bv_e != 0))

    bq_f = bq + bz_e[None, :] * gq      # [T, S]
    bk_f = bk + bz_e[None, :] * gk

    keeps = [np.where(~mask[b])[0] for b in range(B)]
    NK = max(1, -(-max(len(kk) for kk in keeps) // P))
    TK = NK * P

    gqT = np.ascontiguousarray(gq.T.astype(NPFP16))
    bqT = np.ascontiguousarray(bq_f.T.astype(NPFP16))

    in_maps = []
    for c in range(N_CORES):
        b, h = c // 2, c % 2
        cols = slice(h * EH, (h + 1) * EH)
        kidx = keeps[b]
        nk = len(kidx)
        xk = np.zeros((TK, D), NPFP16)
        xk[:nk] = x[b][kidx].astype(NPFP16)
        gkT = np.zeros((S, TK), NPFP16)
        gkT[:, :nk] = gk[kidx].T.astype(NPFP16)
        bkT = np.zeros((S, TK), NPFP16)
        bkT[:, :nk] = bk_f[kidx].T.astype(NPFP16)
        m = {
            "x": np.ascontiguousarray(x[b].astype(NPFP16)),
            "xT": np.ascontiguousarray(x[b].T.astype(NPFP16)),
            "xk": xk,
            "xkT": np.ascontiguousarray(xk.T),
            "gqT": gqT,
            "bqT": bqT,
            "gkT": gkT,
            "bkT": bkT,
            "Wz": np.ascontiguousarray(Wz_c),
            "Wu": np.ascontiguousarray(Wu_c[:, cols]),
            "Wv": np.ascontiguousarray(Wv_c[:, cols]),
            "Wo": np.ascontiguousarray(Wo[cols, :].astype(NPFP16)),
        }
        if with_bu:
            m["bu"] = np.ascontiguousarray(bu_e[cols])
        if with_bv:
            m["bv"] = np.ascontiguousarray(
                bv_e[cols].reshape(1, EH).astype(NPFP16))
        in_maps.append(m)
    return in_maps, NK, with_bu, with_bv


def combine_outputs(inputs, parts):
    """parts: list of 8 [T, D] fp16 partials -> full [B, T, D] fp32."""
    f = np.float32
    x = np.asarray(inputs["x"], f)
    bo = np.asarray(inputs["bo"], f)
    out = np.empty((B, T, D), f)
    for b in range(B):
        out[b] = (parts[2 * b].astype(f) + parts[2 * b + 1].astype(f)
                  + bo[None, :] + x[b])
    return out


_NC_CACHE = {}


def run(inputs, trace=False, **kw):
    in_maps, NK, with_bu, with_bv = prep_core_inputs(inputs)
    key = (NK, with_bu, with_bv)
    if key not in _NC_CACHE:
        _NC_CACHE[key] = build_module(NK, with_bu, with_bv)
    nc = _NC_CACHE[key]
    res = run_bass_kernel_spmd(
        nc, in_maps, core_ids=list(range(N_CORES)), trace=trace, **kw
    )
    parts = [r["outp"] for r in res.results]
    return combine_outputs(inputs, parts), res


def kernel(**inputs):
    out, _ = run(inputs)
    return out
